# revision 1
# baseline (speedup 1.0000x reference)
"""Trainium2 Bass kernel for nn_AttentionLayer (B=4, T=2048, C=1024, H=16).

Sharding (8 cores): core c = (batch b = c//2, head-group g = c%2).
Data parallel on batch, tensor parallel on heads: each core computes the
qkv projection for its 8 heads, causal flash-attention, and a partial
output projection (row split of w_proj). Host sums the two partials per
batch and re-transposes.

Per-core kernel (Bass/Tile, bf16 matmuls, fp32 PSUM accumulation):
  phase A: qkv projection.  Q^T/K^T produced in [head_dim, t] layout
           (moving operand = x^T), V in natural [t, head_dim] layout
           (moving operand = w_v^T) with an appended ones column.
  phase B: causal attention per head-pair.  S^T = K^T.T @ Q^T row-tiled
           2 heads/matmul (contraction 64 x 2), exp on ACT (no
           max-subtract needed: logits are O(1)), causal mask by DVE
           multiply with a constant triangle tile (diagonal tiles are
           width-narrowed to the causal boundary), O^T = [V|1].T @ P^T
           accumulated in PSUM; row 64 gives softmax denominators;
           normalize via DVE reciprocal + K=1 selector matmul
           (partition broadcast on PE).
  phase C: out^T = w_p^T.T @ y^T + bias (bias only on g=0 cores).
  Phases are software-pipelined: emission interleaves projection chunks
  with pending attention/output chunks so the in-order PE stream always
  has matmul filler during exp stalls.

All weights are loaded into SBUF once, outside the steady-state loop
(weights-resident regime); per-iteration DRAM traffic is x in + out out,
both bf16. Output partials are bf16; the host sums the two per-batch
partials in fp32.
"""
from contextlib import ExitStack

import numpy as np
import ml_dtypes

import concourse.bacc as bacc
import concourse.mybir as mybir
import concourse.tile as tile
from concourse.bass_utils import run_bass_kernel_spmd

F32 = mybir.dt.float32
F32R = mybir.dt.float32r
BF16 = mybir.dt.bfloat16
AF = mybir.ActivationFunctionType
NP_BF16 = ml_dtypes.bfloat16

B, T, C, H = 4, 2048, 1024, 16
HD = C // H          # 64
NH = H // 2          # heads per core: 8
QCOLS = NH * HD      # 512


def build(T=T, C=C, NH=NH, HD=HD, TQ=512, loop_iters=1, exp_cols=None,
          skip_mask=False, act_func="exp", act_scale=True, av_flip=False,
          av_depth=2, ldw_probe=False, norm_bcast=False, ratio=3, unroll=1):
    assert C % 128 == 0 and T % TQ == 0 and TQ % 128 == 0
    NP = NH // 2              # head pairs
    CT = C // 128             # contraction tiles
    NTB = T // TQ             # time blocks
    TT = T // 128             # tk tiles
    NO = C // 128             # out row tiles
    QC = NH * HD
    scale = 1.0 / (HD ** 0.5)

    nc = bacc.Bacc()
    xT = nc.declare_dram_parameter("xT", [CT, NTB, 128, TQ], BF16, isOutput=False)
    wqkT = nc.declare_dram_parameter("wqkT", [2, CT, 128, QC], BF16, isOutput=False)
    wvT = nc.declare_dram_parameter("wvT", [CT, 128, QC], BF16, isOutput=False)
    wpT = nc.declare_dram_parameter("wpT", [NP, 128, C], BF16, isOutput=False)
    bias = nc.declare_dram_parameter("bias", [128, NO], F32, isOutput=False)
    outT = nc.declare_dram_parameter("outT", [NTB, 128, NO * TQ], BF16, isOutput=True)

    with tile.TileContext(nc) as tc, ExitStack() as ctx:
        # long-lived pools first (stack allocator)
        qt_pool = ctx.enter_context(tc.tile_pool(name="qt", bufs=NP * NTB))
        kt_pool = ctx.enter_context(tc.tile_pool(name="kt", bufs=NP * NTB))
        v_pool = ctx.enter_context(tc.tile_pool(name="v", bufs=TT))
        wp_pool = ctx.enter_context(tc.tile_pool(name="wp", bufs=NP))
        wqk_pool = ctx.enter_context(tc.tile_pool(name="wqk", bufs=2 * CT))
        wv_pool = ctx.enter_context(tc.tile_pool(name="wv", bufs=CT))
        bias_pool = ctx.enter_context(tc.tile_pool(name="bias", bufs=1))

        bias_sb = bias_pool.tile([128, NO], F32, tag="bias", name="bias_sb")
        nc.sync.dma_start(bias_sb[:], bias[:])
        ones_sb = bias_pool.tile([128, NH], BF16, tag="ones", name="ones_sb")
        nc.gpsimd.memset(ones_sb[:], 1.0)
        # causal band mask: mask_a[x,y]=1 iff y>=x (applied to the first 128
        # columns of every diagonal tile; tiles are narrowed to the boundary)
        ii = np.arange(128)[:, None]
        mask_a_np = (np.arange(128)[None, :] >= ii).astype(NP_BF16)
        sel_np = np.ones((1, 64), np.float32)
        mask_a_dram = nc.inline_tensor(mask_a_np, name="mask_a")
        sel_dram = nc.inline_tensor(sel_np, name="sel")
        mask_a = bias_pool.tile([128, 128], BF16, tag="mask_a", name="mask_a_sb")
        sel_sb = bias_pool.tile([1, 64], F32R, tag="sel", name="sel_sb")
        nc.gpsimd.dma_start(mask_a[:], mask_a_dram[:])
        nc.gpsimd.dma_start(sel_sb[:], sel_dram[:])
        if av_flip:
            ident_dram = nc.inline_tensor(np.eye(128, dtype=NP_BF16), name="ident")
            ident_sb = bias_pool.tile([128, 128], BF16, tag="ident", name="ident_sb")
            nc.gpsimd.dma_start(ident_sb[:], ident_dram[:])
        # resident weights: w_proj, w_qk, w_v all loaded once
        wp_sb = [wp_pool.tile([128, C], BF16, tag="wp", name="wp") for p in range(NP)]
        for p in range(NP):
            nc.sync.dma_start(wp_sb[p][:], wpT[p])
        wqk_sb = [[wqk_pool.tile([128, QC], BF16, tag="wqk", name="wqk")
                   for c in range(CT)] for half in range(2)]
        for half in range(2):
            for c in range(CT):
                nc.sync.dma_start(wqk_sb[half][c][:], wqkT[half, c])
        wv_sb = [wv_pool.tile([128, QC], BF16, tag="wv", name="wv") for c in range(CT)]
        for c in range(CT):
            nc.sync.dma_start(wv_sb[c][:], wvT[c])

        qt = {}
        kt = {}
        vt = []
        yt = {}

        # PSUM budget is 8 banks of 2KB/partition.  Separate rings so the
        # long-lived attention accumulators (oacc) never block transient
        # projection tiles (ops): st 2x2 banks + oacc 2x1 + ops 2x1 = 8.
        # All pools are opened once, OUTSIDE the steady-state loop: per-body
        # pool open/close emits engine drains every iteration.
        st_pool = ctx.enter_context(tc.tile_pool(name="st", bufs=2, space="PSUM"))
        oacc_pool = ctx.enter_context(tc.tile_pool(name="oacc", bufs=2, space="PSUM"))
        o_ps_pool = ctx.enter_context(tc.tile_pool(name="ops", bufs=2, space="PSUM"))
        pt_pool = ctx.enter_context(tc.tile_pool(name="pt",
                                                 bufs=18 if av_flip else 5))
        rc_pool = ctx.enter_context(tc.tile_pool(name="rc", bufs=4))
        osb_pool = ctx.enter_context(tc.tile_pool(name="osb", bufs=2))
        xs_pool = ctx.enter_context(tc.tile_pool(name="xs", bufs=2 * CT))

        def body():
            qt.clear(); kt.clear(); vt.clear(); yt.clear()

            def gen_proj_block(tb, xs):
                """Phase A chunk generator: yields after each matmul group.

                Order: the 4 Q pairs, then K pair 0, then all 4 V tiles, then
                K pairs 1-3 — so this block's attention can start after chunk
                5 and its V dependencies arrive just ahead of their consumers.
                """
                def qk_chunk(half, jp):
                    jt = half * NP + jp
                    ps = o_ps_pool.tile([128, TQ], F32, tag="ops", name="mm")
                    for c in range(CT):
                        wsl = (wqk_sb[half][0][:, 0:128] if ldw_probe else
                               wqk_sb[half][c][:, 128 * jp:128 * (jp + 1)])
                        nc.tensor.matmul(ps[:], wsl,
                                         xs[c][:], start=(c == 0), stop=(c == CT - 1))
                    dst = qt_pool.tile([128, TQ], BF16, tag="qt", name="qt") if jt < NP else kt_pool.tile([128, TQ], BF16, tag="kt", name="kt")
                    nc.vector.tensor_copy(dst[:], ps[:])
                    if jt < NP:
                        qt[(jt, tb)] = dst
                    else:
                        kt[(jt - NP, tb)] = dst

                def v_chunk(ti):
                    tt_i = tb * (TQ // 128) + ti
                    ps = o_ps_pool.tile([128, QC], F32, tag="ops", name="mmv")
                    for c in range(CT):
                        xsl = (xs[0][:, 0:128] if ldw_probe else
                               xs[c][:, 128 * ti:128 * (ti + 1)])
                        nc.tensor.matmul(ps[:], xsl, wv_sb[c][:],
                                         start=(c == 0), stop=(c == CT - 1))
                    vtile = v_pool.tile([128, NH * (HD + 1)], BF16, tag="v", name="v")
                    v3 = vtile[:].rearrange("p (h d) -> p h d", d=HD + 1)
                    nc.vector.tensor_copy(v3[:, :, 0:HD], ps[:].rearrange("p (h d) -> p h d", d=HD))
                    nc.vector.tensor_copy(v3[:, :, HD], ones_sb[:])
                    assert len(vt) == tt_i
                    vt.append(vtile)

                for jp in range(NP):
                    qk_chunk(0, jp)
                    yield
                for step in (("k", 0), ("v", 0), ("v", 1), ("v", 2),
                             ("v", 3), ("k", 1), ("k", 2), ("k", 3)):
                    if step[0] == "k":
                        qk_chunk(1, step[1])
                    else:
                        v_chunk(step[1])
                    yield

            def gen_attention_block(qi):
                """Phase B generator (all pairs, one query block) + phase C."""
                tq0 = qi * TQ
                ntk = (tq0 + TQ) // 128
                def emit_norm(state):
                    p_, o0_, o1_ = state
                    ytile = qt_pool.tile([128, TQ], BF16, tag="qt", name="y")
                    yt[(p_, qi)] = ytile
                    rcA = rc_pool.tile([1, TQ], F32R, tag="rc", name="rcA")
                    rcB = rc_pool.tile([1, TQ], F32R, tag="rcb", name="rcB")
                    with nc.allow_low_precision(reason="denominators kept full fp32"):
                        nc.vector.reciprocal(rcA[:], o0_[HD:HD + 1, :])
                        nc.vector.reciprocal(rcB[:], o1_[HD:HD + 1, :])
                    if norm_bcast:
                        # partition-broadcast of the reciprocal row: one fused
                        # DVE multiply per head straight out of PSUM
                        with nc.allow_low_precision(reason="y kept in bf16 for the PE"):
                            nc.vector.tensor_mul(ytile[0:64, :], o0_[0:HD, :],
                                                 rcA[:].broadcast_to((64, TQ)))
                            nc.vector.tensor_mul(ytile[64:128, :], o1_[0:HD, :],
                                                 rcB[:].broadcast_to((64, TQ)))
                    else:
                        bc0 = o_ps_pool.tile([HD, TQ], F32, tag="ops", name="bc0")
                        bc1 = o_ps_pool.tile([HD, TQ], F32, tag="ops", name="bc1")
                        nc.tensor.matmul(bc0[:], sel_sb[:], rcA[:], start=True, stop=True)
                        nc.tensor.matmul(bc1[:], sel_sb[:], rcB[:], start=True, stop=True)
                        with nc.allow_low_precision(reason="y kept in bf16 for the PE"):
                            nc.scalar.activation(ytile[0:64, :], o0_[0:HD, :], AF.Copy)
                            nc.scalar.activation(ytile[64:128, :], o1_[0:HD, :], AF.Copy)
                            nc.vector.tensor_mul(ytile[0:64, :], ytile[0:64, :], bc0[:])
                            nc.vector.tensor_mul(ytile[64:128, :], ytile[64:128, :], bc1[:])

                pending_norm = None
                for p in range(NP):
                    h0 = 2 * p
                    h1 = 2 * p + 1
                    o0 = oacc_pool.tile([HD + 1, TQ], F32, tag="oacc", name="ops")
                    o1 = oacc_pool.tile([HD + 1, TQ], F32, tag="oacc", name="ops2")

                    def emit_av(state):
                        pt_, w_, dlt_, tki_ = state
                        vtile = vt[0] if ldw_probe else vt[tki_]
                        v3 = vtile[:].rearrange("p (h d) -> p h d", d=HD + 1)
                        va = v3[:, 0, :] if ldw_probe else v3[:, h0, :]
                        vb = v3[:, 0, :] if ldw_probe else v3[:, h1, :]
                        nc.tensor.matmul(o0[:, dlt_:TQ], va, pt_[:, 0:w_],
                                         start=(tki_ == 0), stop=(tki_ == ntk - 1))
                        nc.tensor.matmul(o1[:, dlt_:TQ], vb, pt_[:, w_:2 * w_],
                                         start=(tki_ == 0), stop=(tki_ == ntk - 1))

                    av_q = deque()
                    for tki in range(ntk):
                        tk0 = tki * 128
                        # diagonal narrowing: only q >= tk0 attends (bf16
                        # matmuls keep full rate at any width)
                        dlt = max(0, tk0 - tq0)
                        w = TQ - dlt
                        diag = tk0 >= tq0
                        ktile = kt[(p, tk0 // TQ)]
                        koff = tk0 % TQ
                        qtile = qt[(p, qi)]
                        st = st_pool.tile([128, 2 * TQ], F32, tag="st", name="st")
                        ka = ktile[0:64, 0:128] if ldw_probe else ktile[0:64, koff:koff + 128]
                        kb = ktile[0:64, 0:128] if ldw_probe else ktile[64:128, koff:koff + 128]
                        qa = qtile[0:64, dlt:TQ]
                        qb = qtile[0:64, dlt:TQ] if ldw_probe else qtile[64:128, dlt:TQ]
                        nc.tensor.matmul(st[:, 0:w], ka, qa, start=True, stop=True)
                        nc.tensor.matmul(st[:, TQ:TQ + w], kb, qb, start=True, stop=True)
                        pt = pt_pool.tile([128, 2 * TQ], BF16, tag="pt", name="pt")
                        ec = w if exp_cols is None else (w // 2 if exp_cols == -2 else exp_cols)
                        st_v = st[:].rearrange("p (h q) -> p h q", q=TQ)[:, :, 0:ec]
                        pt_v = pt[:, 0:2 * w].rearrange("p (h q) -> p h q", h=2)[:, :, 0:ec]
                        af = AF.Exp if act_func == "exp" else AF.Copy
                        with nc.allow_low_precision(reason="attention probs in bf16"):
                            if act_scale:
                                nc.scalar.activation(pt_v, st_v, af, scale=scale)
                            else:
                                nc.scalar.activation(pt_v, st_v, af)
                        if diag and not skip_mask:
                            band = pt[:, 0:2 * w].rearrange("p (h q) -> p h q", h=2)[:, :, 0:128]
                            with nc.allow_low_precision(reason="attention probs in bf16"):
                                nc.vector.tensor_mul(band, band,
                                                     mask_a[:].rearrange("p q -> p () q").broadcast_to((128, 2, 128)))
                        # one-deep rotation: AV(i-1) is emitted after S(i) so
                        # the PE always has the next S-pair during exp stalls;
                        # the previous segment's normalize is likewise deferred
                        # past this segment's first S/exp
                        if tki == 0 and pending_norm is not None:
                            emit_norm(pending_norm)
                            pending_norm = None
                            yield
                        if len(av_q) >= av_depth:
                            emit_av(av_q.popleft())
                            yield
                        av_q.append((pt, w, dlt, tki))
                    while av_q:
                        emit_av(av_q.popleft())
                        if av_q:
                            yield
                    pending_norm = (p, o0, o1)
                    yield
                    yield
                if pending_norm is not None:
                    emit_norm(pending_norm)
                    pending_norm = None
                    yield
                # phase C for this time block: accumulate all NO output tiles
                # into one SBUF buffer, then store with a single DMA
                tb = qi
                osb = osb_pool.tile([128, NO * TQ], BF16, tag="osb", name="osb")
                for ot in range(NO):
                    ps = o_ps_pool.tile([128, TQ], F32, tag="ops", name="mmo")
                    for p in range(NP):
                        wpsl = (wp_sb[0][:, 0:128] if ldw_probe else
                                wp_sb[p][:, 128 * ot:128 * (ot + 1)])
                        nc.tensor.matmul(ps[:], wpsl, yt[(p, tb)][:],
                                         start=(p == 0), stop=(p == NP - 1))
                    with nc.allow_low_precision(reason="output partials in bf16"):
                        nc.vector.tensor_scalar_add(osb[:, ot * TQ:(ot + 1) * TQ], ps[:],
                                                    bias_sb[:, ot:ot + 1])
                    yield
                nc.sync.dma_start(outT[tb], osb[:])

            def gen_attention_block_flip(qi):
                """Phase B with flipped AV: o[q, d] accumulated per q-subtile
                with stationary P, per-partition normalize, PE transpose back
                to [feature, q] for the output projection."""
                tq0 = qi * TQ
                ntk = (tq0 + TQ) // 128
                for p in range(NP):
                    h0 = 2 * p
                    h1 = 2 * p + 1
                    ytile = qt_pool.tile([128, TQ], BF16, tag="qt", name="y")
                    yt[(p, qi)] = ytile
                    pts = []

                    def emit_s_exp(tki):
                        tk0 = tki * 128
                        dlt = max(0, tk0 - tq0)
                        w = TQ - dlt
                        diag = tk0 >= tq0
                        ktile = kt[(p, tk0 // TQ)]
                        koff = tk0 % TQ
                        qtile = qt[(p, qi)]
                        st = st_pool.tile([128, 2 * TQ], F32, tag="st", name="st")
                        ka = ktile[0:64, 0:128] if ldw_probe else ktile[0:64, koff:koff + 128]
                        kb = ktile[0:64, 0:128] if ldw_probe else ktile[64:128, koff:koff + 128]
                        qa = qtile[0:64, dlt:TQ]
                        qb = qtile[0:64, dlt:TQ] if ldw_probe else qtile[64:128, dlt:TQ]
                        nc.tensor.matmul(st[:, 0:w], ka, qa, start=True, stop=True)
                        nc.tensor.matmul(st[:, TQ:TQ + w], kb, qb, start=True, stop=True)
                        pt = pt_pool.tile([128, 2 * TQ], BF16, tag="pt", name="pt")
                        st_v = st[:].rearrange("p (h q) -> p h q", q=TQ)[:, :, 0:w]
                        pt_v = pt[:, 0:2 * w].rearrange("p (h q) -> p h q", h=2)
                        with nc.allow_low_precision(reason="attention probs in bf16"):
                            nc.scalar.activation(pt_v, st_v, AF.Exp, scale=scale)
                        if diag:
                            band = pt[:, 0:2 * w].rearrange("p (h q) -> p h q", h=2)[:, :, 0:128]
                            with nc.allow_low_precision(reason="attention probs in bf16"):
                                nc.vector.tensor_mul(band, band,
                                                     mask_a[:].rearrange("p q -> p () q").broadcast_to((128, 2, 128)))
                        pts.append((pt, w))

                    navail = 0
                    for qs in range(4):
                        need = 4 * qi + qs + 1
                        target = min(need + 1, ntk)
                        while navail < target:
                            emit_s_exp(navail)
                            navail += 1
                            yield
                        o = oacc_pool.tile([128, 2 * (HD + 1)], F32, tag="oacc", name="oacc")
                        for tki in range(need):
                            ptile, w = pts[tki]
                            colA = qs * 128 - (TQ - w)
                            v3 = vt[tki][:].rearrange("p (h d) -> p h d", d=HD + 1)
                            nc.tensor.matmul(o[:, 0:HD + 1], ptile[:, colA:colA + 128],
                                             v3[:, h0, :], start=(tki == 0),
                                             stop=(tki == need - 1), skip_group_check=True)
                            nc.tensor.matmul(o[:, HD + 1:2 * (HD + 1)],
                                             ptile[:, w + colA:w + colA + 128],
                                             v3[:, h1, :], start=(tki == 0),
                                             stop=(tki == need - 1), skip_group_check=True)
                            if tki % 8 == 7:
                                yield
                        yield
                        rc = rc_pool.tile([128, 2], F32, tag="rc2", name="rc")
                        with nc.allow_low_precision(reason="denominators in fp32"):
                            nc.vector.reciprocal(rc[:], o[:, HD:2 * HD + 2:HD + 1])
                        ysb = rc_pool.tile([128, 128], BF16, tag="ysb", name="ysb")
                        with nc.allow_low_precision(reason="y in bf16"):
                            nc.vector.tensor_scalar_mul(ysb[:, 0:HD], o[:, 0:HD], rc[:, 0:1])
                            nc.vector.tensor_scalar_mul(ysb[:, HD:2 * HD],
                                                        o[:, HD + 1:2 * HD + 1], rc[:, 1:2])
                        ytp = o_ps_pool.tile([128, 128], BF16, tag="ops", name="ytp")
                        nc.tensor.transpose(ytp[:], ysb[:], ident_sb[:])
                        with nc.allow_low_precision(reason="y in bf16"):
                            nc.scalar.activation(ytile[:, qs * 128:(qs + 1) * 128],
                                                 ytp[:], AF.Copy)
                        yield
                # phase C for this time block
                tb = qi
                osb = osb_pool.tile([128, NO * TQ], BF16, tag="osb", name="osb")
                for ot in range(NO):
                    ps = o_ps_pool.tile([128, TQ], F32, tag="ops", name="mmo")
                    for p in range(NP):
                        wpsl = (wp_sb[0][:, 0:128] if ldw_probe else
                                wp_sb[p][:, 128 * ot:128 * (ot + 1)])
                        nc.tensor.matmul(ps[:], wpsl, yt[(p, tb)][:],
                                         start=(p == 0), stop=(p == NP - 1))
                    with nc.allow_low_precision(reason="output partials in bf16"):
                        nc.vector.tensor_scalar_add(osb[:, ot * TQ:(ot + 1) * TQ], ps[:],
                                                    bias_sb[:, ot:ot + 1])
                    yield
                nc.sync.dma_start(outT[tb], osb[:])

            gen_att = gen_attention_block_flip if av_flip else gen_attention_block

            # software pipeline: interleave phase-A chunks of block tb with
            # pending phase-B/C chunks so the PE instruction stream always
            # has projection matmuls to fill attention stalls.
            from collections import deque
            pending = deque()

            def step_att():
                while pending:
                    try:
                        next(pending[0])
                        return True
                    except StopIteration:
                        pending.popleft()
                return False

            def issue_xs(tb):
                xs = [xs_pool.tile([128, TQ], BF16, tag="xs", name="xs")
                      for _ in range(CT)]
                for c in range(CT):
                    nc.sync.dma_start(xs[c][:], xT[c, tb])
                return xs

            xs_next = issue_xs(0)
            for tb in range(NTB):
                xs_cur = xs_next
                for ci, _ in enumerate(gen_proj_block(tb, xs_cur)):
                    if ci == 4 and tb + 1 < NTB:
                        xs_next = issue_xs(tb + 1)
                    for _ in range(ratio):
                        step_att()
                pending.append(gen_att(tb))
            while pending:
                step_att()

        if loop_iters == 1:
            body()
        elif unroll == 2 and loop_iters % 2 == 0:
            # two bodies per hardware-loop iteration: halves the per-layer
            # cost of the loop back-edge engine drains
            with tc.For_i(0, loop_iters // 2, 1):
                body()
                body()
        else:
            with tc.For_i(0, loop_iters, 1):
                body()
    nc.finalize()
    return nc


def _tile2d(a, pr, pc):
    """[R, S] -> [R//pr, S//pc, pr, pc] contiguous tiles."""
    R, S = a.shape
    return np.ascontiguousarray(
        a.reshape(R // pr, pr, S // pc, pc).transpose(0, 2, 1, 3))


def shard_inputs(x, w_attn, w_proj, b_proj, TQ=512):
    """Returns in_maps for 8 cores: core c = (b=c//2, g=c%2)."""
    CT = C // 128
    NP = NH // 2
    wq, wk, wv = w_attn[0:C], w_attn[C:2 * C], w_attn[2 * C:3 * C]
    x = np.asarray(x)
    in_maps = []
    for core in range(8):
        b = core // 2
        g = core % 2
        rows = slice(g * QCOLS, (g + 1) * QCOLS)
        xTt = _tile2d(np.asarray(x[b]).T, 128, TQ)                 # [CT,NTB,128,TQ]
        wqkT_flat = np.concatenate([wq[rows], wk[rows]], 0).T      # [C, 2QC]
        wqkTt = np.ascontiguousarray(
            wqkT_flat.reshape(CT, 128, 2, QCOLS).transpose(2, 0, 1, 3))  # [2,CT,128,QC]
        wvTt = np.ascontiguousarray(wv[rows].T.reshape(CT, 128, QCOLS))
        wpTt = np.ascontiguousarray(w_proj[:, rows].T.reshape(NP, 128, C))
        in_maps.append({
            "xT": xTt.astype(NP_BF16),
            "wqkT": wqkTt.astype(NP_BF16),
            "wvT": wvTt.astype(NP_BF16),
            "wpT": wpTt.astype(NP_BF16),
            "bias": (np.ascontiguousarray(b_proj.reshape(C // 128, 128).T)
                     if g == 0 else np.zeros((128, C // 128), np.float32)),
        })
    return in_maps


def unshard_output(outT_tiles_pair, TQ=512):
    """outT [NTB,128,NO*TQ] bf16 partials (2 cores) -> out [T, C] fp32."""
    s = (outT_tiles_pair[0].astype(np.float32)
         + outT_tiles_pair[1].astype(np.float32))
    NO, NTB = C // 128, T // TQ
    s = s.reshape(NTB, 128, NO, TQ).transpose(2, 1, 0, 3)  # [NO,128,NTB,TQ]
    return s.reshape(C, T).T


_NC_CACHE = {}


def kernel(x, w_attn, w_proj, b_proj):
    if "nc" not in _NC_CACHE:
        _NC_CACHE["nc"] = build()
    nc = _NC_CACHE["nc"]
    in_maps = shard_inputs(x, w_attn, w_proj, b_proj)
    res = run_bass_kernel_spmd(nc, in_maps, core_ids=list(range(8)))
    out = np.empty((B, T, C), np.float32)
    for b in range(B):
        out[b] = unshard_output([res.results[2 * b]["outT"],
                                 res.results[2 * b + 1]["outT"]])
    return out

